# revision 38
# baseline (speedup 1.0000x reference)
"""ClassAttention (decode-style single-query attention) on 8 TRN2 NeuronCores.

Math (per batch b):
    kv = x @ Wkv              # [N, 2*H*D], k half cols 0:1024, v half 1024:2048
    q  = x[0] @ Wq            # [H*D]  (CLS token only)
    logits[t, h] = scale * sum_d q[h,d] * k[t, h*64+d]
    attn = softmax_t(logits)
    cls[h,d] = sum_t attn[t,h] * v[t, h*64+d]
    out = cls @ Wproj + bproj

Key restructuring (v2):
  - k is never materialized: logits = x @ wk_fold, with
    wk_fold[c,h] = scale * sum_d q[h,d] * Wk[c, h*64+d]   (folded per batch).
  - v is never materialized either: attention output is reassociated as
    xaT[c,h] = sum_t exp[t,h] * x[t,c]  (accumulated TRANSPOSED on the PE
    with x's natural layout as the moving operand and exp[128t,16h] as the
    tiny stationary), then cls[h,:] = diag-pick of (xaT_n @ Wv).
  - Softmax runs without max-subtraction (logits are O(1) by construction)
    and the 1/sum(exp) normalization is deferred to the tiny cls tensor.
  - sum_t exp[t,h] rides the same PSUM accumulation as xaT via a ones-column
    matmul (no vector work in the sweep).

Pipelining (the point of v2):
  - ALL DMA cast-loads (f32->bf16) go on the single SWDGE (gpsimd) queue in
    bandwidth-optimal order: Wq, Wk first (they gate the q-fold), then batch
    0's x groups, then Wv/Wproj (needed only at finalize), then batch 1's x.
    Loads self-gate on pool buffers and stream continuously.
  - X-bar transposes (for the logits operand x^T) all on the sync queue,
    arrival-gated, overlapping everything.
  - The PE queue is software-pipelined with depth 2: the attention-accumulate
    matmuls of tile i are emitted after the logits matmuls of tile i+2, so
    the in-order PE queue never stalls on the Scalar EXP dependency.

Sharding: pure data-parallel over B: 16 batches / 8 cores = 2 per core.
Weights are replicated; each core returns its [2, 1024] output shard.
"""

import numpy as np

import concourse.bass as bass
import concourse.mybir as mybir
import concourse.tile as tile
from concourse import bacc
from concourse.bass_utils import run_bass_kernel_spmd
from concourse.masks import make_identity

F32 = mybir.dt.float32
BF16 = mybir.dt.bfloat16

B, SEQ, C = 16, 4096, 1024
H, D = 16, 64
SCALE = D ** -0.5  # 0.125
N_CORES = 8
BPC = B // N_CORES          # batches per core
T_TILES = SEQ // 128        # 32 sequence tiles of 128 rows per batch
CB = C // 128               # 8 contraction blocks
GRP = 4                     # t-tiles per DMA group
NG = T_TILES // GRP         # 8 groups per batch


def _build():
    nc = bacc.Bacc(
        "TRN2", target_bir_lowering=False, debug=False, num_devices=N_CORES
    )
    x_ap = nc.dram_tensor("x", [BPC, SEQ, C], F32, kind="ExternalInput").ap()
    wq_ap = nc.dram_tensor("Wq", [C, H * D], F32, kind="ExternalInput").ap()
    wkv_ap = nc.dram_tensor("Wkv", [C, 2 * H * D], F32, kind="ExternalInput").ap()
    wp_ap = nc.dram_tensor("Wproj", [H * D, C], F32, kind="ExternalInput").ap()
    bp_ap = nc.dram_tensor("bproj", [C], F32, kind="ExternalInput").ap()
    out_ap = nc.dram_tensor("out", [BPC, C], F32, kind="ExternalOutput").ap()

    with tile.TileContext(nc) as tc:
        _emit(nc, tc, x_ap, wq_ap, wkv_ap, wp_ap, bp_ap, out_ap)
    nc.compile()
    return nc


def _emit(nc, tc, x_ap, wq_ap, wkv_ap, wp_ap, bp_ap, out_ap):
    with tc.tile_pool(name="consts", bufs=1) as consts:
        wq_bf = consts.tile([128, CB * 1024], BF16)
        wk_bf = consts.tile([128, CB * 1024], BF16)
        wv_bf = consts.tile([128, CB * 1024], BF16)
        wp_bf = consts.tile([128, CB * 1024], BF16)

        bproj_sb = consts.tile([1, C], F32)
        nc.scalar.dma_start(bproj_sb[:], bp_ap[:].unsqueeze(0))

        # CLS rows of x, transposed on load: xcls[p, b*CB+g] = x[b, 0, g*128+p]
        xcls_bf = consts.tile([128, BPC * CB], BF16)
        for b in range(BPC):
            nc.gpsimd.dma_start(
                xcls_bf[:, b * CB : (b + 1) * CB],
                x_ap[b, 0:1, :].rearrange("o (g p) -> p (o g)", p=128),
            )

        # ---- SWDGE cast-load order: Wq, Wk | x b0 | x b1 ----
        def load_w(dst, src_ap, c0):
            for g in range(CB):
                nc.gpsimd.dma_start(
                    dst[:, g * 1024 : (g + 1) * 1024],
                    src_ap[g * 128 : (g + 1) * 128, c0 : c0 + 1024],
                )

        load_w(wq_bf, wq_ap, 0)
        load_w(wk_bf, wkv_ap, 0)

        ones_bf = consts.tile([128, 128], BF16)      # sums-of-exp matmul lhsT
        nc.vector.memset(ones_bf[:], 1.0)
        sc_row = consts.tile([1, 128], BF16)         # scale * ones: q broadcast
        nc.vector.memset(sc_row[:], SCALE)
        identity = consts.tile([16, 16], F32)        # PE-transpose identity (fin)
        make_identity(nc, identity[:])
        id128 = consts.tile([128, 128], BF16)        # PE-transpose identity (x)
        make_identity(nc, id128[:])

        with (
            tc.tile_pool(name="xbf", bufs=8) as xbf_pool,
            tc.tile_pool(name="xt", bufs=3) as xt_pool,
            tc.tile_pool(name="wstage", bufs=2) as wstage,
            tc.tile_pool(name="fold", bufs=2) as fold_pool,
            tc.tile_pool(name="persist", bufs=1) as persist,
            tc.tile_pool(name="small", bufs=2) as small,
            tc.tile_pool(name="exp", bufs=4) as exp_pool,
        ):
            # Wv/Wproj: only needed at finalize. Load f32 on the sync HWDGE
            # queue (concurrent with the SWDGE stream) and cast on the DVE.
            for dst, src_ap, c0 in ((wv_bf, wkv_ap, 1024), (wp_bf, wp_ap, 0)):
                for g in range(CB):
                    wst = wstage.tile([128, 1024], F32, tag="wst")
                    nc.sync.dma_start(
                        wst[:],
                        src_ap[g * 128 : (g + 1) * 128, c0 : c0 + 1024],
                    )
                    # cast on the ACT engine: scalar is idle until the sweep
                    nc.scalar.copy(dst[:, g * 1024 : (g + 1) * 1024], wst[:])

            # ---- all x group loads (gpsimd, buffer-gated) ----
            xbf_tiles = {}
            for b in range(BPC):
                for tg in range(NG):
                    x_bf = xbf_pool.tile([128, GRP * C], BF16, tag="xbf")
                    nc.gpsimd.dma_start(
                        x_bf[:].rearrange("p (i c) -> p i c", i=GRP),
                        x_ap[b, tg * GRP * 128 : (tg + 1) * GRP * 128, :].rearrange(
                            "(i t) c -> t i c", i=GRP
                        ),
                    )
                    xbf_tiles[(b, tg)] = x_bf

            # ---- q + wk_fold per batch ----
            wkfs = []
            with tc.tile_pool(name="qpsum", bufs=1, space="PSUM") as qpsum:
                for b in range(BPC):
                    wkfs.append(
                        _emit_qfold(
                            nc, b, qpsum, xcls_bf, wq_bf, wk_bf, sc_row,
                            fold_pool, persist, small,
                        )
                    )

            # ---- sweeps + finalize ----
            with (
                tc.tile_pool(name="lgpsum", bufs=2, space="PSUM") as lgpsum,
                tc.tile_pool(name="raps", bufs=2, space="PSUM") as raps,
                tc.tile_pool(name="smps", bufs=1, space="PSUM") as smps,
                tc.tile_pool(name="trpsum", bufs=2, space="PSUM") as trpsum,
                tc.tile_pool(name="finpsum", bufs=1, space="PSUM") as finpsum,
            ):
                for b in range(BPC):
                    rA, rB, sm = _emit_sweep(
                        nc, b, wkfs[b], ones_bf, id128, xbf_tiles,
                        lgpsum, raps, smps, trpsum, xt_pool, exp_pool, persist,
                    )
                    _emit_finalize(
                        nc, b, rA, rB, sm, wv_bf, wp_bf, bproj_sb, id128,
                        small, finpsum, trpsum, out_ap,
                    )


def _emit_qfold(nc, b, qpsum, xcls_bf, wq_bf, wk_bf, sc_row, fold_pool, persist, small):
    """q = x_cls @ Wq; wk_fold[c,h] = scale * sum_d q[h,d]*Wk[c,(h,d)]."""
    q_ps = qpsum.tile([1, H * D], F32, tag="qps")
    for g in range(CB):
        lt = xcls_bf[:, b * CB + g : b * CB + g + 1]
        for ch in range(2):
            nc.tensor.matmul(
                q_ps[0:1, ch * 512 : (ch + 1) * 512],
                lt,
                wq_bf[:, g * 1024 + ch * 512 : g * 1024 + (ch + 1) * 512],
                start=(g == 0),
                stop=(g == CB - 1),
            )
    q_sb = small.tile([1, H * D], BF16, tag="qsb")
    nc.vector.tensor_copy(q_sb[:], q_ps[:])

    # qb[c_p, hd] = scale * q[hd]  (outer product broadcast down partitions)
    qb_ps = qpsum.tile([128, H * D], F32, tag="qbps")
    for ch in range(2):
        nc.tensor.matmul(
            qb_ps[:, ch * 512 : (ch + 1) * 512],
            sc_row[0:1, :],
            q_sb[0:1, ch * 512 : (ch + 1) * 512],
            start=True,
            stop=True,
        )
    qb_sb = persist.tile([128, H * D], BF16, tag=f"qb{b}")
    nc.vector.tensor_copy(qb_sb[:], qb_ps[:])

    wkf_bf = persist.tile([128, CB * H], BF16, tag=f"wkf{b}")
    for g in range(CB):
        prod = fold_pool.tile([128, H * D], BF16, tag="prod")
        nc.vector.tensor_mul(
            prod[:], wk_bf[:, g * 1024 : (g + 1) * 1024], qb_sb[:]
        )
        wkf_g = fold_pool.tile([128, H], F32, tag="wkfg")
        nc.vector.tensor_reduce(
            wkf_g[:].unsqueeze(2),
            prod[:].rearrange("p (h d) -> p h d", d=D),
            axis=mybir.AxisListType.X,
            op=mybir.AluOpType.add,
        )
        nc.vector.tensor_copy(wkf_bf[:, g * H : (g + 1) * H], wkf_g[:])
    return wkf_bf


def _emit_sweep(nc, b, wkf_bf, ones_bf, id128, xbf_tiles, lgpsum, raps, smps,
                trpsum, xt_pool, exp_pool, persist):
    """logits -> exp -> PSUM-chained attention accumulate, PE-pipelined.

    r[h, c] accumulates in two PSUM banks (chains of 32 matmuls each) and
    sum_t exp rides the same stationary e block as one extra N=1 matmul,
    so no vector-engine adds sit in the sweep or tail."""
    rA = raps.tile([16, 512], F32, tag="ra", name="rA")
    rB = raps.tile([16, 512], F32, tag="ra", name="rB")
    sm = smps.tile([16, 512], F32, tag="sm", name="sm")

    def emit_ptrans(x_bf):
        """x^T for the logits GEMM, on the PE (the DMA pipe is the scarce
        resource; X-bar transposes double its traffic).  PSUM->SBUF copies
        alternate between the vector and scalar engines."""
        xt = xt_pool.tile([128, GRP * CB * 128], BF16, tag="xt")
        for k in range(GRP * CB // 4):
            tps = trpsum.tile([128, 512], BF16, tag="tps")
            for j in range(4):
                blk = k * 4 + j
                nc.tensor.transpose(
                    tps[:, j * 128 : (j + 1) * 128],
                    x_bf[:, blk * 128 : (blk + 1) * 128],
                    id128[:],
                )
            if k % 2 == 0:
                nc.vector.tensor_copy(xt[:, k * 512 : (k + 1) * 512], tps[:])
            else:
                nc.scalar.copy(xt[:, k * 512 : (k + 1) * 512], tps[:])
        return xt

    def emit_xat(e, x_bf, i, ti):
        first = ti == 0
        last = ti == T_TILES - 1
        nc.tensor.matmul(
            rA[:], e[:], x_bf[:, i * C : i * C + 512],
            start=first, stop=last,
        )
        nc.tensor.matmul(
            rB[:], e[:], x_bf[:, i * C + 512 : (i + 1) * C],
            start=first, stop=last,
        )
        nc.tensor.matmul(
            sm[0:16, 0:1], e[:], ones_bf[:, 0:1],
            start=first, stop=last,
        )

    pending = []
    xt = emit_ptrans(xbf_tiles[(b, 0)])
    for tg in range(NG):
        x_bf, cur_xt = xbf_tiles[(b, tg)], xt
        for i in range(GRP):
            ti = tg * GRP + i
            lg = lgpsum.tile([128, H], F32, tag="lg")
            for g in range(CB):
                nc.tensor.matmul(
                    lg[:],
                    cur_xt[:, (i * CB + g) * 128 : (i * CB + g + 1) * 128],
                    wkf_bf[:, g * H : (g + 1) * H],
                    start=(g == 0), stop=(g == CB - 1),
                )
            e = exp_pool.tile([128, H], BF16, tag="exp")
            nc.scalar.activation(e[:], lg[:], mybir.ActivationFunctionType.Exp)
            pending.append((e, x_bf, i, ti))
            if len(pending) > 2:
                emit_xat(*pending.pop(0))
        if tg + 1 < NG:
            xt = emit_ptrans(xbf_tiles[(b, tg + 1)])
    for args in pending:
        emit_xat(*args)
    return rA, rB, sm


def _emit_finalize(nc, b, rA, rB, sm, wv_bf, wp_bf, bproj_sb, id128, small,
                   finpsum, trpsum, out_ap):
    """cls = diag((r/S)^T-chains @ Wv), out = cls @ Wproj + bproj."""
    sums = small.tile([16, 1], F32, tag="sums")
    nc.vector.tensor_copy(sums[:], sm[0:16, 0:1])
    rec = small.tile([16, 1], F32, tag="rec")
    nc.vector.reciprocal(rec[:], sums[:])
    r_bf = small.tile([16, C], BF16, tag="rbf")
    nc.vector.tensor_scalar_mul(r_bf[:, 0:512], rA[:], rec[:])
    nc.vector.tensor_scalar_mul(r_bf[:, 512:1024], rB[:], rec[:])

    rT_ps = trpsum.tile([128, 512], BF16, tag="tps")
    for g in range(CB):
        nc.tensor.transpose(
            rT_ps[:, g * H : (g + 1) * H],
            r_bf[:, g * 128 : (g + 1) * 128],
            id128[0:16, 0:16],
        )
    rT_bf = small.tile([128, 128], BF16, tag="rTb")
    nc.vector.tensor_copy(rT_bf[:], rT_ps[:, 0:128])

    cls_bf = small.tile([16, C], BF16, tag="cls_sb")
    for ch in range(2):
        cls_ps = finpsum.tile([16, 512], F32, tag="fin2")
        for g in range(CB):
            nc.tensor.matmul(
                cls_ps[:],
                rT_bf[:, g * H : (g + 1) * H],
                wv_bf[:, g * 1024 + ch * 512 : g * 1024 + (ch + 1) * 512],
                start=(g == 0), stop=(g == CB - 1),
            )
        nc.vector.tensor_copy(cls_bf[:, ch * 512 : (ch + 1) * 512], cls_ps[:])

    # diagonal pick: clsv[hd] = cls_bf[hd//64, hd]
    aT = trpsum.tile([128, 512], BF16, tag="tps")
    for g in range(CB):
        nc.tensor.transpose(
            aT[:, g * H : (g + 1) * H],
            cls_bf[:, g * 128 : (g + 1) * 128],
            id128[0:16, 0:16],
        )
    clsv = small.tile([128, CB], BF16, tag="cls_bf")
    for g in range(CB):
        for half in range(2):
            rows = slice(64 * half, 64 * half + 64)
            col = g * H + 2 * g + half
            nc.vector.tensor_copy(clsv[rows, g : g + 1], aT[rows, col : col + 1])

    # out = clsv @ Wproj + bproj
    o_sb = small.tile([1, C], F32, tag="osb")
    for ch in range(2):
        o_ps = finpsum.tile([16, 512], F32, tag="fin2")
        for g in range(CB):
            nc.tensor.matmul(
                o_ps[0:1, :],
                clsv[:, g : g + 1],
                wp_bf[:, g * 1024 + ch * 512 : g * 1024 + (ch + 1) * 512],
                start=(g == 0), stop=(g == CB - 1),
            )
        nc.vector.tensor_add(
            o_sb[0:1, ch * 512 : (ch + 1) * 512], o_ps[0:1, :],
            bproj_sb[0:1, ch * 512 : (ch + 1) * 512],
        )
    nc.sync.dma_start(out_ap[b : b + 1, :], o_sb[:])


_CACHED = None


def _get_program():
    global _CACHED
    if _CACHED is None:
        _CACHED = _build()
    return _CACHED


def kernel(x, Wq, Wkv, Wproj, bproj, _trace=False):
    x = np.ascontiguousarray(np.asarray(x, dtype=np.float32))
    Wq = np.ascontiguousarray(np.asarray(Wq, dtype=np.float32))
    Wkv = np.ascontiguousarray(np.asarray(Wkv, dtype=np.float32))
    Wproj = np.ascontiguousarray(np.asarray(Wproj, dtype=np.float32))
    bproj = np.ascontiguousarray(np.asarray(bproj, dtype=np.float32))

    nc = _get_program()
    in_maps = [
        {
            "x": x[cid * BPC : (cid + 1) * BPC],
            "Wq": Wq,
            "Wkv": Wkv,
            "Wproj": Wproj,
            "bproj": bproj,
        }
        for cid in range(N_CORES)
    ]
    res = run_bass_kernel_spmd(
        nc, in_maps, core_ids=list(range(N_CORES)), trace=_trace
    )
    out = np.concatenate([res.results[cid]["out"] for cid in range(N_CORES)], axis=0)
    if _trace:
        kernel.last_exec_time_ns = res.exec_time_ns
        kernel.last_results = res
    return out.reshape(B, 1, C)

